# revision 1
# baseline (speedup 1.0000x reference)
"""Bass/Trainium2 kernel for HCFC-GNN (3-layer GCN + hierarchy max-constraint).

Strategy (8 NeuronCores, SPMD):
  - Nodes sharded 6250/core. Edges (incl. self-loops) sharded by TARGET core,
    sorted by (target block, source half).
  - GCN norm folded into the table:  out[c] = dinv[c] * (sum_{e->c} g[row_e] + ...),
    with g = dinv * (h @ W^T + b). Bias rides inside the table; self-loops are
    plain edges.
  - Per layer: shard dense transform (PE) -> AllGather bf16 table (shard-strided
    6272-row chunks; zero pad rows usable as gather padding) -> per 128-node
    block: dma_gather source rows (two int16-safe halves of 25088 rows), build
    one-hot S via DVE is_equal against an iota row, scatter-add via PE matmul
    S^T @ M accumulated in PSUM.
  - Final: sigmoid, then out[n,i] = max_j R[i,j]*h[n,j] via DVE mult+reduce_max.
"""

import os
import numpy as np
import ml_dtypes

N = 50000
E = 1600000
C = 13
DIN = 12
H = 128
NCORES = 8
SH = N // NCORES          # 6250 nodes per shard
CH = 6272                 # shard chunk rows in gathered table (6250 + 22 zero pad)
BLK = (SH + 127) // 128   # 49 blocks per shard (last block 106 nodes)
LASTB = SH - (BLK - 1) * 128  # 106
HALF = 4 * CH             # 25088 rows per gather half (int16-safe)
ZROW = SH                 # local zero-row index inside each half (= first pad row)
PADCREL = 300.0           # colrel value guaranteed not to match iota 0..127

bf16 = ml_dtypes.bfloat16

LAST_RESULTS = None


def _prep_edges(edge_index):
    """Partition/sort edges; build per-core gather-index and colrel streams with
    block/half slot sizes (TL) uniform across cores so one SPMD program works."""
    row = np.concatenate([edge_index[0], np.arange(N, dtype=np.int32)])
    col = np.concatenate([edge_index[1], np.arange(N, dtype=np.int32)])
    deg = np.bincount(row, minlength=N).astype(np.float32)

    s_shard = row // SH
    grow = s_shard * CH + (row % SH)       # row index in gathered table [0, 8*CH)
    half = (grow >= HALF).astype(np.int64)
    gloc = np.where(half == 0, grow, grow - HALF).astype(np.int64)
    tcore = col // SH
    tcol = col % SH
    blk = tcol // 128
    crel = (tcol % 128).astype(np.int64)

    key = ((tcore * BLK) + blk) * 2 + half
    order = np.lexsort((gloc, key))
    key_s = key[order]
    gloc_s = gloc[order]
    crel_s = crel[order]

    nslots = NCORES * BLK * 2
    cnt = np.bincount(key_s, minlength=nslots).reshape(NCORES, BLK, 2)
    starts = np.zeros(nslots + 1, np.int64)
    np.cumsum(cnt.reshape(-1), out=starts[1:])

    # uniform tile counts across cores
    TL = np.maximum(1, ((cnt + 127) // 128).max(axis=0))  # [BLK, 2]
    off = np.zeros((BLK, 2), np.int64)                    # slot offsets in tiles
    tot = [0, 0]
    for h in (0, 1):
        for b in range(BLK):
            off[b, h] = tot[h]
            tot[h] += TL[b, h]

    gidx = []   # per core: (gidx_lo, gidx_hi, crel_lo, crel_hi)
    for k in range(NCORES):
        per_half = []
        for h in (0, 1):
            gparts, cparts = [], []
            for b in range(BLK):
                s = starts[(k * BLK + b) * 2 + h]
                e = starts[(k * BLK + b) * 2 + h + 1]
                n = int(e - s)
                m = int(TL[b, h]) * 128
                gseg = np.full(m, ZROW, np.int64)
                cseg = np.full(m, PADCREL, np.float64)
                gseg[:n] = gloc_s[s:e]
                cseg[:n] = crel_s[s:e]
                # wrapped idx layout: pos i -> partition i%16, col i//16
                gparts.append(gseg.reshape(m // 16, 16).T.astype(np.int16))
                # colrel layout: pos i -> partition i%128, col i//128
                cparts.append(cseg.reshape(m // 128, 128).T.astype(bf16))
            g = np.hstack(gparts)                      # [16, tot_h*8]
            per_half.append((np.tile(g, (8, 1)).copy(), np.hstack(cparts).copy()))
        gidx.append(per_half)
    return deg, TL, off, tot, gidx


def _build_program(TL, off):
    import concourse.bacc as bacc
    import concourse.mybir as mybir
    import concourse.tile as tile

    dt = mybir.dt
    nc = bacc.Bacc("TRN2", target_bir_lowering=False, debug=False,
                   num_devices=NCORES)

    # inputs
    xs = nc.dram_tensor("xs", [DIN, CH], dt.float32, kind="ExternalInput")
    degs = nc.dram_tensor("degs", [128, BLK], dt.float32, kind="ExternalInput")
    W1T = nc.dram_tensor("W1T", [DIN, H], dt.float32, kind="ExternalInput")
    b1r = nc.dram_tensor("b1r", [1, H], dt.float32, kind="ExternalInput")
    W2T = nc.dram_tensor("W2T", [H, H], dt.bfloat16, kind="ExternalInput")
    b2r = nc.dram_tensor("b2r", [1, H], dt.bfloat16, kind="ExternalInput")
    W3T = nc.dram_tensor("W3T", [H, H], dt.bfloat16, kind="ExternalInput")
    b3r = nc.dram_tensor("b3r", [1, H], dt.bfloat16, kind="ExternalInput")
    Rfl = nc.dram_tensor("Rfl", [128, C * C], dt.float32, kind="ExternalInput")
    iota_in = nc.dram_tensor("iota_in", [128, 128], dt.bfloat16, kind="ExternalInput")
    ident_in = nc.dram_tensor("ident_in", [128, 128], dt.float32, kind="ExternalInput")
    ones_f = nc.dram_tensor("ones_f", [1, 128], dt.float32, kind="ExternalInput")
    ones_b = nc.dram_tensor("ones_b", [1, 128], dt.bfloat16, kind="ExternalInput")
    gi_lo = nc.dram_tensor("gi_lo", [128, 8 * int(off[-1, 0] + TL[-1, 0])], dt.int16,
                           kind="ExternalInput")
    gi_hi = nc.dram_tensor("gi_hi", [128, 8 * int(off[-1, 1] + TL[-1, 1])], dt.int16,
                           kind="ExternalInput")
    cr_lo = nc.dram_tensor("cr_lo", [128, int(off[-1, 0] + TL[-1, 0])], dt.bfloat16,
                           kind="ExternalInput")
    cr_hi = nc.dram_tensor("cr_hi", [128, int(off[-1, 1] + TL[-1, 1])], dt.bfloat16,
                           kind="ExternalInput")
    out = nc.dram_tensor("out", [SH, C], dt.float32, kind="ExternalOutput")

    gin = nc.dram_tensor("gin", [CH, H], dt.bfloat16)
    gout = nc.dram_tensor("gout", [NCORES * CH, H], dt.bfloat16,
                          addr_space="Shared")
    gpriv = nc.dram_tensor("gpriv", [NCORES * CH, H], dt.bfloat16)

    TOT = [int(off[-1, 0] + TL[-1, 0]), int(off[-1, 1] + TL[-1, 1])]
    TLMAX = int(TL.max())

    with tile.TileContext(nc) as tc:
        with (
            tc.tile_pool(name="const", bufs=1) as cpool,
            tc.tile_pool(name="idx", bufs=1) as ipool,
            tc.tile_pool(name="msg", bufs=6) as mpool,
            tc.tile_pool(name="sbl", bufs=6) as spool,
            tc.tile_pool(name="hblk", bufs=3) as hpool,
            tc.tile_pool(name="gblk", bufs=3) as gpool,
            tc.tile_pool(name="psum", bufs=3, space="PSUM") as pp,
            tc.tile_pool(name="psumt", bufs=2, space="PSUM") as ppt,
        ):
            # ---- constants ----
            xs_t = cpool.tile([DIN, CH], dt.float32)
            nc.sync.dma_start(out=xs_t[:], in_=xs[:])
            w1_t = cpool.tile([DIN, H], dt.float32)
            nc.sync.dma_start(out=w1_t[:], in_=W1T[:])
            b1_t = cpool.tile([1, H], dt.float32)
            nc.sync.dma_start(out=b1_t[:], in_=b1r[:])
            w2_t = cpool.tile([H, H], dt.bfloat16)
            nc.sync.dma_start(out=w2_t[:], in_=W2T[:])
            b2_t = cpool.tile([1, H], dt.bfloat16)
            nc.sync.dma_start(out=b2_t[:], in_=b2r[:])
            w3_t = cpool.tile([H, H], dt.bfloat16)
            nc.sync.dma_start(out=w3_t[:], in_=W3T[:])
            b3_t = cpool.tile([1, H], dt.bfloat16)
            nc.sync.dma_start(out=b3_t[:], in_=b3r[:])
            r_t = cpool.tile([128, C * C], dt.float32)
            nc.sync.dma_start(out=r_t[:], in_=Rfl[:])
            io_t = cpool.tile([128, 128], dt.bfloat16)
            nc.sync.dma_start(out=io_t[:], in_=iota_in[:])
            id_t = cpool.tile([128, 128], dt.float32)
            nc.sync.dma_start(out=id_t[:], in_=ident_in[:])
            of_t = cpool.tile([1, 128], dt.float32)
            nc.sync.dma_start(out=of_t[:], in_=ones_f[:])
            ob_t = cpool.tile([1, 128], dt.bfloat16)
            nc.sync.dma_start(out=ob_t[:], in_=ones_b[:])
            gil_t = ipool.tile([128, 8 * TOT[0]], dt.int16)
            nc.sync.dma_start(out=gil_t[:], in_=gi_lo[:])
            gih_t = ipool.tile([128, 8 * TOT[1]], dt.int16)
            nc.sync.dma_start(out=gih_t[:], in_=gi_hi[:])
            crl_t = ipool.tile([128, TOT[0]], dt.bfloat16)
            nc.sync.dma_start(out=crl_t[:], in_=cr_lo[:])
            crh_t = ipool.tile([128, TOT[1]], dt.bfloat16)
            nc.sync.dma_start(out=crh_t[:], in_=cr_hi[:])

            # dinv = 1/sqrt(deg) on device
            deg_t = cpool.tile([128, BLK], dt.float32)
            nc.sync.dma_start(out=deg_t[:], in_=degs[:])
            sq_t = cpool.tile([128, BLK], dt.float32)
            nc.scalar.sqrt(sq_t[:], deg_t[:])
            dinv_t = cpool.tile([128, BLK], dt.float32)
            nc.vector.reciprocal(dinv_t[:], sq_t[:])

            # zero tail of gin (rows SH..CH)
            z_t = cpool.tile([32, H], dt.bfloat16)
            nc.vector.memset(z_t[:], 0.0)
            nc.sync.dma_start(out=gin[SH:CH, :], in_=z_t[0:CH - SH, :])

            halves = ((gil_t, crl_t, gpriv[0:HALF, :]),
                      (gih_t, crh_t, gpriv[HALF:2 * HALF, :]))

            def agg_block(b, width):
                """Gather+scatter for node block b; returns PSUM tile [128,width]."""
                acc = pp.tile([128, width], dt.float32, tag="aggpsum")
                first = True
                for h in (0, 1):
                    gi_t, cr_t, src = halves[h]
                    tl = int(TL[b, h])
                    o = int(off[b, h])
                    msg = mpool.tile([128, TLMAX, H], dt.bfloat16, tag="msg")
                    nc.gpsimd.dma_gather(
                        out_ap=msg[:, 0:tl, :], in_ap=src,
                        idxs_ap=gi_t[:, o * 8:(o + tl) * 8],
                        num_idxs=tl * 128, num_idxs_reg=tl * 128, elem_size=H,
                        single_packet=False,
                    )
                    S = spool.tile([128, TLMAX, 128], dt.bfloat16, tag="sb")
                    nc.vector.tensor_tensor(
                        out=S[:, 0:tl, :],
                        in0=cr_t[:, o:o + tl].unsqueeze(2).broadcast_to([128, tl, 128]),
                        in1=io_t[:, :].unsqueeze(1).broadcast_to([128, tl, 128]),
                        op=mybir.AluOpType.is_equal,
                    )
                    for j in range(tl):
                        last = (h == 1 and j == int(TL[b, 1]) - 1)
                        nc.tensor.matmul(acc[:, :], S[:, j, :], msg[:, j, 0:width],
                                         start=first, stop=last)
                        first = False
                return acc

            def transform_and_gin(b, hblk_bf):
                """table row block = (h @ W^T + b) for layer l; hblk_bf is
                [128,128] bf16 transposed input (features on partitions)."""
                pass

            # ---------------- Layer 1 transform: g1 = dinv*(x@W1T + b1) -----
            for b in range(BLK):
                acc = ppt.tile([128, H], dt.float32, tag="tfpsum")
                nc.tensor.matmul(acc[:, :], xs_t[:, b * 128:b * 128 + 128],
                                 w1_t[:, :], start=True, stop=False)
                nc.tensor.matmul(acc[:, :], of_t[:, :], b1_t[:, :],
                                 start=False, stop=True)
                g = gpool.tile([128, H], dt.bfloat16, tag="g")
                nc.vector.tensor_scalar_mul(g[:, :], acc[:, :], dinv_t[:, b:b + 1])
                nc.sync.dma_start(out=gin[b * 128:b * 128 + 128, :], in_=g[:, :])

            nc.gpsimd.collective_compute(
                "AllGather", mybir.AluOpType.bypass,
                replica_groups=[list(range(NCORES))],
                ins=[gin[:, :]], outs=[gout[:, :]],
            )
            nc.sync.dma_start(out=gpriv[:, :], in_=gout[:, :])

            # ---------------- Layers 2,3: agg -> h -> transform -> allgather
            for lyr, (wt, bt) in ((2, (w2_t, b2_t)), (3, (w3_t, b3_t))):
                for b in range(BLK):
                    acc = agg_block(b, H)
                    hblk = hpool.tile([128, H], dt.float32, tag="h")
                    nc.scalar.activation(hblk[:, :], acc[:, :],
                                         mybir.ActivationFunctionType.Relu,
                                         scale=dinv_t[:, b:b + 1])
                    tp = ppt.tile([128, H], dt.float32, tag="tp")
                    nc.tensor.transpose(tp[:, :], hblk[:, :], id_t[:, :])
                    htb = hpool.tile([128, H], dt.bfloat16, tag="htb")
                    nc.vector.tensor_copy(htb[:, :], tp[:, :])
                    acc2 = ppt.tile([128, H], dt.float32, tag="tfpsum")
                    nc.tensor.matmul(acc2[:, :], htb[:, :], wt[:, :],
                                     start=True, stop=False)
                    nc.tensor.matmul(acc2[:, :], ob_t[:, :], bt[:, :],
                                     start=False, stop=True)
                    g = gpool.tile([128, H], dt.bfloat16, tag="g")
                    nc.vector.tensor_scalar_mul(g[:, :], acc2[:, :],
                                                dinv_t[:, b:b + 1])
                    nc.sync.dma_start(out=gin[b * 128:b * 128 + 128, :], in_=g[:, :])
                nc.gpsimd.collective_compute(
                    "AllGather", mybir.AluOpType.bypass,
                    replica_groups=[list(range(NCORES))],
                    ins=[gin[:, :]], outs=[gout[:, :]],
                )
                nc.sync.dma_start(out=gpriv[:, :], in_=gout[:, :])

            # ---------------- final agg + sigmoid + hierarchy max ----------
            for b in range(BLK):
                acc = agg_block(b, 16)
                h3 = hpool.tile([128, 16], dt.float32, tag="h3")
                nc.scalar.activation(h3[:, :], acc[:, :],
                                     mybir.ActivationFunctionType.Sigmoid,
                                     scale=dinv_t[:, b:b + 1])
                tmp = hpool.tile([128, C, C], dt.float32, tag="tmp")
                nc.vector.tensor_tensor(
                    out=tmp[:, :, :],
                    in0=h3[:, 0:C].unsqueeze(1).broadcast_to([128, C, C]),
                    in1=r_t[:, :].rearrange("p (a b) -> p a b", a=C),
                    op=mybir.AluOpType.mult,
                )
                o13 = gpool.tile([128, C], dt.float32, tag="o13")
                nc.vector.tensor_reduce(o13[:, :], tmp[:, :, :],
                                        axis=mybir.AxisListType.X,
                                        op=mybir.AluOpType.max)
                rows = 128 if b < BLK - 1 else LASTB
                nc.sync.dma_start(out=out[b * 128:b * 128 + rows, :],
                                  in_=o13[0:rows, :])

    nc.compile()
    return nc


def kernel(x, edge_index, R, W1, b1, W2, b2, W3, b3, **_):
    global LAST_RESULTS
    import concourse.mybir  # noqa: F401  (ensure env importable early)
    from concourse.bass_utils import run_bass_kernel_spmd

    x = np.asarray(x, np.float32)
    edge_index = np.asarray(edge_index, np.int32)
    deg, TL, off, tot, gidx = _prep_edges(edge_index)

    nc = _build_program(TL, off)

    # common inputs
    W1T = np.ascontiguousarray(np.asarray(W1, np.float32).T)
    b1r = np.asarray(b1, np.float32)[None, :]
    W2T = np.ascontiguousarray(np.asarray(W2, np.float32).T.astype(bf16))
    b2r = np.asarray(b2, np.float32).astype(bf16)[None, :]
    W3Tp = np.zeros([H, H], bf16)
    W3Tp[:, :C] = np.asarray(W3, np.float32).T.astype(bf16)
    b3r = np.zeros([1, H], bf16)
    b3r[0, :C] = np.asarray(b3, np.float32).astype(bf16)
    Rfl = np.tile(np.asarray(R, np.float32).reshape(1, C * C), (128, 1))
    iota = np.tile(np.arange(128, dtype=np.float32).astype(bf16), (128, 1))
    ident = np.eye(128, dtype=np.float32)
    ones_f = np.ones([1, 128], np.float32)
    ones_b = np.ones([1, 128], bf16)

    in_maps = []
    for k in range(NCORES):
        xs = np.zeros([DIN, CH], np.float32)
        xs[:, :SH] = x[k * SH:(k + 1) * SH].T
        degs = np.ones([BLK * 128], np.float32)
        degs[:SH] = deg[k * SH:(k + 1) * SH]
        degs = np.ascontiguousarray(degs.reshape(BLK, 128).T)
        (g_lo, c_lo), (g_hi, c_hi) = gidx[k]
        in_maps.append({
            "xs": xs, "degs": degs, "W1T": W1T, "b1r": b1r, "W2T": W2T,
            "b2r": b2r, "W3T": W3Tp, "b3r": b3r, "Rfl": Rfl, "iota_in": iota,
            "ident_in": ident, "ones_f": ones_f, "ones_b": ones_b,
            "gi_lo": g_lo, "gi_hi": g_hi, "cr_lo": c_lo, "cr_hi": c_hi,
        })

    trace = os.environ.get("GNN_TRACE") == "1"
    res = run_bass_kernel_spmd(nc, in_maps, core_ids=list(range(NCORES)),
                               trace=trace)
    LAST_RESULTS = res

    reps = int(os.environ.get("GNN_BENCH", "0"))
    if reps > 0:
        _bench(nc, in_maps, reps)
    return np.concatenate([res.results[k]["out"] for k in range(NCORES)], axis=0)


BENCH_TIMES = None
BENCH_PIPELINED_NS = None


def _bench(nc, in_maps, reps):
    """Time repeated executions of the already-built program through a single
    jit instance (NEFF compile amortized away; inputs device_put once)."""
    global BENCH_TIMES
    import time
    import jax
    import numpy as jnp_np
    from jax.sharding import Mesh, PartitionSpec, NamedSharding
    from jax.experimental.shard_map import shard_map
    import concourse.mybir as mybir
    from concourse.bass2jax import (_bass_exec_p, partition_id_tensor,
                                    install_neuronx_cc_hook)

    install_neuronx_cc_hook()
    in_names, out_names, out_avals, zero_outs = [], [], [], []
    pname = nc.partition_id_tensor.name if nc.partition_id_tensor else None
    for alloc in nc.m.functions[0].allocations:
        if not isinstance(alloc, mybir.MemoryLocationSet):
            continue
        name = alloc.memorylocations[0].name
        if alloc.kind == "ExternalInput":
            if name != pname:
                in_names.append(name)
        elif alloc.kind == "ExternalOutput":
            out_names.append(name)
            shape = tuple(alloc.tensor_shape)
            dtype = mybir.dt.np(alloc.dtype)
            out_avals.append(jax.core.ShapedArray(shape, dtype))
            zero_outs.append(np.zeros(shape, dtype))
    n_params = len(in_names)
    all_names = in_names + out_names + ([pname] if pname else [])

    def _body(*args):
        ops = list(args)
        if pname:
            ops.append(partition_id_tensor())
        return tuple(_bass_exec_p.bind(
            *ops, out_avals=tuple(out_avals), in_names=tuple(all_names),
            out_names=tuple(out_names), lowering_input_output_aliases=(),
            sim_require_finite=True, sim_require_nnan=True, nc=nc))

    devices = jax.devices()[:NCORES]
    mesh = Mesh(np.asarray(devices), ("core",))
    nouts = len(out_names)
    sharded = jax.jit(
        shard_map(_body, mesh=mesh,
                  in_specs=(PartitionSpec("core"),) * (n_params + nouts),
                  out_specs=(PartitionSpec("core"),) * nouts, check_rep=False),
        donate_argnums=tuple(range(n_params, n_params + nouts)),
        keep_unused=True)
    sh = NamedSharding(mesh, PartitionSpec("core"))
    dev_in = [jax.device_put(
        np.concatenate([np.asarray(in_maps[c][nm]) for c in range(NCORES)], axis=0), sh)
        for nm in in_names]
    times = []
    for i in range(reps + 1):
        zs = [jax.device_put(
            np.zeros((NCORES * z.shape[0], *z.shape[1:]), z.dtype), sh)
            for z in zero_outs]
        t0 = time.perf_counter()
        outs = sharded(*dev_in, *zs)
        jax.block_until_ready(outs)
        times.append(time.perf_counter() - t0)
    BENCH_TIMES = times
    print("bench wall times (s):", " ".join(f"{t:.4f}" for t in times))
    print(f"bench min/median after warmup: {min(times[1:]):.4f} / "
          f"{sorted(times[1:])[len(times[1:]) // 2]:.4f}")

    # pipelined async dispatch: amortizes per-call RPC overhead
    NPIPE = 20
    zss = [[jax.device_put(
        np.zeros((NCORES * z.shape[0], *z.shape[1:]), z.dtype), sh)
        for z in zero_outs] for _ in range(NPIPE)]
    t0 = time.perf_counter()
    outs = None
    for i in range(NPIPE):
        outs = sharded(*dev_in, *zss[i])
    jax.block_until_ready(outs)
    tp = (time.perf_counter() - t0) / NPIPE
    global BENCH_PIPELINED_NS
    BENCH_PIPELINED_NS = int(tp * 1e9)
    print(f"bench pipelined per-exec: {tp * 1e3:.3f} ms "
          f"({tp * 1e9:.0f} ns upper bound)")



# revision 2
# speedup vs baseline: 1.0186x; 1.0186x over previous
"""Bass/Trainium2 kernel for HCFC-GNN (3-layer GCN + hierarchy max-constraint).

Strategy (8 NeuronCores, SPMD, pull-mode with target-sharded edges):
  - Nodes sharded 6250/core; edges (incl. self-loops) sharded by TARGET core,
    sorted by (target block, source half). Self loops ride as plain edges.
  - Layer-1 aggregation operates on the PRE-transform features: the table is
    t1 = dinv * [x | 1] (host-built, bf16, 16 cols in 128-col rows), so the
    L1 dense transform folds in AFTER aggregation via Wk = [W1^T; b1]. This
    kills one on-device transform pass + one AllGather.
  - GCN norm folding: raw agg A[c] = sum_{e->c} t[src]; per node n the next
    table row is t' = dinv_n^2 * (relu(A_n) @ W^T) + dinv_n * b, realized as
    matmul + rank-1 bias (sqrtdeg_n outer b) + one per-partition scale.
  - Scatter-add via PE one-hot matmul. L1/L2 run "swapped" (stationary=msg,
    streamed=S) so the accumulator lands feature-major, which is exactly the
    lhsT the following dense transform wants -> no per-block PE transpose.
    L3 runs un-swapped so the final [node,16] lands node-major for
    sigmoid + R-max directly.
  - Gathers read straight from the AllGather output (no private copy) and
    round-robin across 4 SWDGE queues.
  - Inputs packed into 4 tensors (x128 table shard, int16 gather indices,
    bf16 blob, f32 blob) -- per-exec dispatch overhead scales with the
    input-tensor list, not with device work.
"""

import os
import numpy as np
import ml_dtypes

N = 50000
E = 1600000
C = 13
DIN = 12
H = 128
NCORES = 8
SH = N // NCORES          # 6250 nodes per shard
CH = 6272                 # shard chunk rows in gathered table (6250 + 22 pad)
BLK = (SH + 127) // 128   # 49 blocks per shard (last block 106 nodes)
LASTB = SH - (BLK - 1) * 128  # 106
HALF = 4 * CH             # 25088 rows per gather half (int16-safe)
ZROW = SH                 # pad row index inside each half
PADCREL = 300.0           # colrel value guaranteed not to match iota 0..127

bf16 = ml_dtypes.bfloat16

LAST_RESULTS = None


def _prep_edges(edge_index):
    """Partition/sort edges; build per-core merged gather-index and colrel
    streams with slot sizes (TL) uniform across cores so one SPMD program
    works. Returns deg, TL, off (abs tile offsets incl. half concat), TOT
    (total tiles lo/hi), and per-core (gi, cr) arrays."""
    row = np.concatenate([edge_index[0], np.arange(N, dtype=np.int32)])
    col = np.concatenate([edge_index[1], np.arange(N, dtype=np.int32)])
    deg = np.bincount(row, minlength=N).astype(np.float32)

    s_shard = row // SH
    grow = s_shard * CH + (row % SH)       # row index in gathered table
    half = (grow >= HALF).astype(np.int64)
    gloc = np.where(half == 0, grow, grow - HALF).astype(np.int64)
    tcore = col // SH
    tcol = col % SH
    blk = tcol // 128
    crel = (tcol % 128).astype(np.int64)

    key = ((tcore * BLK) + blk) * 2 + half
    order = np.lexsort((gloc, key))
    key_s = key[order]
    gloc_s = gloc[order]
    crel_s = crel[order]

    nslots = NCORES * BLK * 2
    cnt = np.bincount(key_s, minlength=nslots).reshape(NCORES, BLK, 2)
    starts = np.zeros(nslots + 1, np.int64)
    np.cumsum(cnt.reshape(-1), out=starts[1:])

    # uniform tile counts across cores
    TL = np.maximum(1, ((cnt + 127) // 128).max(axis=0))  # [BLK, 2]
    TOT = [int(TL[:, 0].sum()), int(TL[:, 1].sum())]
    off = np.zeros((BLK, 2), np.int64)   # absolute tile offset in merged gi
    tot = [0, 0]
    for hh in (0, 1):
        for b in range(BLK):
            off[b, hh] = tot[hh] + (TOT[0] if hh == 1 else 0)
            tot[hh] += TL[b, hh]

    gidx = []   # per core: (gi [128, 8*TT] int16, cr [128, TT] bf16)
    for k in range(NCORES):
        gparts, cparts = [], []
        for hh in (0, 1):
            for b in range(BLK):
                s = starts[(k * BLK + b) * 2 + hh]
                e = starts[(k * BLK + b) * 2 + hh + 1]
                n = int(e - s)
                m = int(TL[b, hh]) * 128
                gseg = np.full(m, ZROW, np.int64)
                cseg = np.full(m, PADCREL, np.float64)
                gseg[:n] = gloc_s[s:e]
                cseg[:n] = crel_s[s:e]
                # wrapped idx layout: pos i -> partition i%16, col i//16
                gparts.append(gseg.reshape(m // 16, 16).T.astype(np.int16))
                # colrel layout: pos i -> partition i%128, col i//128
                cparts.append(cseg.reshape(m // 128, 128).T.astype(bf16))
        g = np.hstack(gparts)                      # [16, TT*8]
        gi = np.ascontiguousarray(np.tile(g, (8, 1)))   # [128, TT*8]
        cr = np.ascontiguousarray(np.hstack(cparts))    # [128, TT]
        gidx.append((gi, cr))
    return deg, TL, off, TOT, gidx


def _build_program(TL, off, TOT):
    import concourse.bacc as bacc
    import concourse.mybir as mybir
    import concourse.tile as tile

    dt = mybir.dt
    nc = bacc.Bacc("TRN2", target_bir_lowering=False, debug=False,
                   num_devices=NCORES, num_swdge_queues=4)

    TT = TOT[0] + TOT[1]
    TLMAX = int(TL.max())
    SDW = BLK * 128          # 6272 cols for sqrtdeg row
    # bf16 blob layout (cols): cr [0:TT], iota, W2cols, W3cols, Wk, sd-region
    C_IO = TT
    C_W2 = C_IO + 128
    C_W3 = C_W2 + 128
    C_WK = C_W3 + 16
    C_SD = C_WK + 128
    CB = C_SD + SDW + 144
    # f32 blob layout: dinv [0:BLK], dinv2, Rfl(169), ident(128)
    F_D2 = BLK
    F_R = 2 * BLK
    F_ID = F_R + C * C
    FB = F_ID + 128

    x128 = nc.dram_tensor("x128", [CH, H], dt.bfloat16, kind="ExternalInput")
    gi = nc.dram_tensor("gi", [128, 8 * TT], dt.int16, kind="ExternalInput")
    crb = nc.dram_tensor("crb", [128, CB], dt.bfloat16, kind="ExternalInput")
    f32b = nc.dram_tensor("f32b", [128, FB], dt.float32, kind="ExternalInput")
    out = nc.dram_tensor("out", [SH, C], dt.float32, kind="ExternalOutput")

    gin = nc.dram_tensor("gin", [CH, H], dt.bfloat16)
    gout_a = nc.dram_tensor("gout_a", [NCORES * CH, H], dt.bfloat16,
                            addr_space="Shared")
    gout_b = nc.dram_tensor("gout_b", [NCORES * CH, H], dt.bfloat16,
                            addr_space="Shared")

    qn = [0]  # gather queue rotation

    with tile.TileContext(nc) as tc:
        with (
            tc.tile_pool(name="const", bufs=1) as cpool,
            tc.tile_pool(name="idx", bufs=1) as ipool,
            tc.tile_pool(name="msg", bufs=6) as mpool,
            tc.tile_pool(name="sbl", bufs=6) as spool,
            tc.tile_pool(name="hblk", bufs=4) as hpool,
            tc.tile_pool(name="gblk", bufs=3) as gpool,
            tc.tile_pool(name="psum", bufs=3, space="PSUM") as pp,
            tc.tile_pool(name="psumt", bufs=3, space="PSUM") as ppt,
        ):
            # ---- loads ----
            gi_t = ipool.tile([128, 8 * TT], dt.int16)
            nc.sync.dma_start(out=gi_t[:], in_=gi[:])
            cr_t = cpool.tile([128, TT], dt.bfloat16)
            nc.sync.dma_start(out=cr_t[:], in_=crb[:, 0:TT])
            io_t = cpool.tile([128, 128], dt.bfloat16)
            nc.sync.dma_start(out=io_t[:], in_=crb[:, C_IO:C_IO + 128])
            w2c = cpool.tile([128, 128], dt.bfloat16)
            nc.sync.dma_start(out=w2c[:], in_=crb[:, C_W2:C_W2 + 128])
            w3c = cpool.tile([128, 16], dt.bfloat16)
            nc.sync.dma_start(out=w3c[:], in_=crb[:, C_W3:C_W3 + 16])
            wk = cpool.tile([128, 128], dt.bfloat16)
            nc.sync.dma_start(out=wk[:], in_=crb[:, C_WK:C_WK + 128])
            sdb = cpool.tile([128, SDW + 144], dt.bfloat16)
            nc.sync.dma_start(out=sdb[:], in_=crb[:, C_SD:C_SD + SDW + 144])
            dinv_t = cpool.tile([128, BLK], dt.float32)
            nc.sync.dma_start(out=dinv_t[:], in_=f32b[:, 0:BLK])
            dinv2_t = cpool.tile([128, BLK], dt.float32)
            nc.sync.dma_start(out=dinv2_t[:], in_=f32b[:, F_D2:F_D2 + BLK])
            r_t = cpool.tile([128, C * C], dt.float32)
            nc.sync.dma_start(out=r_t[:], in_=f32b[:, F_R:F_R + C * C])
            id_t = cpool.tile([128, 128], dt.float32)
            nc.sync.dma_start(out=id_t[:], in_=f32b[:, F_ID:F_ID + 128])

            # L1 table shard -> gin, AllGather
            nc.sync.dma_start(out=gin[:, :], in_=x128[:, :])
            nc.gpsimd.collective_compute(
                "AllGather", mybir.AluOpType.bypass,
                replica_groups=[list(range(NCORES))],
                ins=[gin[:, :]], outs=[gout[:, :]],
            )

            halves = (gout[0:HALF, :], gout[HALF:2 * HALF, :])

            def agg_block(b, width, swapped):
                """Gather + scatter for node block b.
                swapped: acc[0:width,0:128] = msg^T @ S (feature-major).
                else:    acc[0:128,0:width] = S^T @ msg (node-major)."""
                acc = pp.tile([128, 128], dt.float32, tag="aggpsum")
                first = True
                for hh in (0, 1):
                    tl = int(TL[b, hh])
                    o = int(off[b, hh])
                    msg = mpool.tile([128, TLMAX, H], dt.bfloat16, tag="msg")
                    nc.gpsimd.dma_gather(
                        out_ap=msg[:, 0:tl, :], in_ap=halves[hh],
                        idxs_ap=gi_t[:, o * 8:(o + tl) * 8],
                        num_idxs=tl * 128, num_idxs_reg=tl * 128, elem_size=H,
                        single_packet=False, queue_num=qn[0],
                    )
                    qn[0] = (qn[0] + 1) % 4
                    S = spool.tile([128, TLMAX, 128], dt.bfloat16, tag="sb")
                    nc.vector.tensor_tensor(
                        out=S[:, 0:tl, :],
                        in0=cr_t[:, o:o + tl].unsqueeze(2)
                            .broadcast_to([128, tl, 128]),
                        in1=io_t[:, :].unsqueeze(1).broadcast_to([128, tl, 128]),
                        op=mybir.AluOpType.is_equal,
                    )
                    for j in range(tl):
                        last = (hh == 1 and j == int(TL[b, 1]) - 1)
                        if swapped:
                            nc.tensor.matmul(acc[0:width, :],
                                             msg[:, j, 0:width], S[:, j, :],
                                             start=first, stop=last)
                        else:
                            nc.tensor.matmul(acc[:, 0:width],
                                             S[:, j, :], msg[:, j, 0:width],
                                             start=first, stop=last)
                        first = False
                return acc

            # ---------------- Layer 1: agg(t1) -> h1 -> t2 ------------------
            for b in range(BLK):
                acc = agg_block(b, 16, swapped=True)
                a1 = hpool.tile([16, 128], dt.bfloat16, tag="a1")
                nc.vector.tensor_copy(a1[:, :], acc[0:16, :])
                h1 = ppt.tile([128, 128], dt.float32, tag="tp")
                nc.tensor.matmul(h1[:, :], a1[:, :], wk[0:16, :],
                                 start=True, stop=True)
                p1 = hpool.tile([128, 128], dt.float32, tag="p1")
                nc.scalar.activation(p1[:, :], h1[:, :],
                                     mybir.ActivationFunctionType.Relu)
                tp = ppt.tile([128, 128], dt.float32, tag="tp")
                nc.tensor.transpose(tp[:, :], p1[:, :],
                                    io_t[:, :])  # identity via f32? see below
                p1b = hpool.tile([128, 128], dt.bfloat16, tag="p1b")
                nc.vector.tensor_copy(p1b[:, :], tp[:, :])
                acc2 = ppt.tile([128, 128], dt.float32, tag="tp")
                nc.tensor.matmul(acc2[:, :], p1b[:, :], w2c[:, :],
                                 start=True, stop=False)
                nc.tensor.matmul(acc2[:, :], sdb[0:1, b * 128:b * 128 + 128],
                                 sdb[0:1, SDW:SDW + 128], start=False,
                                 stop=True)
                g2 = gpool.tile([128, 128], dt.bfloat16, tag="g")
                nc.vector.tensor_scalar_mul(g2[:, :], acc2[:, :],
                                            dinv2_t[:, b:b + 1])
                nc.sync.dma_start(out=gin[b * 128:b * 128 + 128, :],
                                  in_=g2[:, :])
            nc.gpsimd.collective_compute(
                "AllGather", mybir.AluOpType.bypass,
                replica_groups=[list(range(NCORES))],
                ins=[gin[:, :]], outs=[gout[:, :]],
            )

            # ---------------- Layer 2: agg(t2) -> h2 -> t3 ------------------
            for b in range(BLK):
                acc = agg_block(b, 128, swapped=True)
                p2 = hpool.tile([128, 128], dt.bfloat16, tag="p1b")
                nc.scalar.activation(p2[:, :], acc[:, :],
                                     mybir.ActivationFunctionType.Relu)
                acc3 = ppt.tile([128, 128], dt.float32, tag="tp")
                nc.tensor.matmul(acc3[:, 0:16], p2[:, :], w3c[:, :],
                                 start=True, stop=False)
                nc.tensor.matmul(acc3[:, 0:16],
                                 sdb[0:1, b * 128:b * 128 + 128],
                                 sdb[0:1, SDW + 128:SDW + 144], start=False,
                                 stop=True)
                g3 = gpool.tile([128, 16], dt.bfloat16, tag="g3")
                nc.vector.tensor_scalar_mul(g3[:, :], acc3[:, 0:16],
                                            dinv2_t[:, b:b + 1])
                nc.sync.dma_start(out=gin[b * 128:b * 128 + 128, 0:16],
                                  in_=g3[:, :])
            nc.gpsimd.collective_compute(
                "AllGather", mybir.AluOpType.bypass,
                replica_groups=[list(range(NCORES))],
                ins=[gin[:, :]], outs=[gout[:, :]],
            )

            # ---------------- Layer 3: agg(t3) -> sigmoid -> R-max ----------
            for b in range(BLK):
                acc = agg_block(b, 16, swapped=False)
                sg = hpool.tile([128, 16], dt.float32, tag="sg")
                nc.scalar.activation(sg[:, :], acc[:, 0:16],
                                     mybir.ActivationFunctionType.Sigmoid,
                                     scale=dinv_t[:, b:b + 1])
                tmp = hpool.tile([128, C, C], dt.float32, tag="tmp")
                nc.vector.tensor_tensor(
                    out=tmp[:, :, :],
                    in0=sg[:, 0:C].unsqueeze(1).broadcast_to([128, C, C]),
                    in1=r_t[:, :].rearrange("p (a b) -> p a b", a=C),
                    op=mybir.AluOpType.mult,
                )
                o13 = gpool.tile([128, C], dt.float32, tag="o13")
                nc.vector.tensor_reduce(o13[:, :], tmp[:, :, :],
                                        axis=mybir.AxisListType.X,
                                        op=mybir.AluOpType.max)
                rows = 128 if b < BLK - 1 else LASTB
                nc.sync.dma_start(out=out[b * 128:b * 128 + rows, :],
                                  in_=o13[0:rows, :])

    nc.compile()
    return nc


def kernel(x, edge_index, R, W1, b1, W2, b2, W3, b3, **_):
    global LAST_RESULTS
    import concourse.mybir  # noqa: F401
    from concourse.bass_utils import run_bass_kernel_spmd

    x = np.asarray(x, np.float32)
    edge_index = np.asarray(edge_index, np.int32)
    deg, TL, off, TOT, gidx = _prep_edges(edge_index)

    nc = _build_program(TL, off, TOT)

    TT = TOT[0] + TOT[1]
    SDW = BLK * 128
    C_IO = TT
    C_W2 = C_IO + 128
    C_W3 = C_W2 + 128
    C_WK = C_W3 + 16
    C_SD = C_WK + 128
    CB = C_SD + SDW + 144
    F_D2 = BLK
    F_R = 2 * BLK
    F_ID = F_R + C * C
    FB = F_ID + 128

    dinv = 1.0 / np.sqrt(deg)
    dinv2 = dinv * dinv
    sqrtdeg = np.sqrt(deg)

    W1 = np.asarray(W1, np.float32)
    W2 = np.asarray(W2, np.float32)
    W3 = np.asarray(W3, np.float32)
    b1 = np.asarray(b1, np.float32)
    b2 = np.asarray(b2, np.float32)
    b3 = np.asarray(b3, np.float32)
    R = np.asarray(R, np.float32)

    # shared bf16 blob pieces
    iota = np.tile(np.arange(128, dtype=np.float32), (128, 1))
    w2cols = W2.T                                   # [H(in f), H(out o)]
    w3cols = np.zeros((128, 16), np.float32)
    w3cols[:, :C] = W3.T
    wkm = np.zeros((128, 128), np.float32)
    wkm[0:DIN, :] = W1.T
    wkm[DIN, :] = b1

    in_maps = []
    for k in range(NCORES):
        nd = slice(k * SH, (k + 1) * SH)
        x128 = np.zeros((CH, H), bf16)
        x128[:SH, 0:DIN] = (x[nd] * dinv[nd][:, None]).astype(bf16)
        x128[:SH, DIN] = dinv[nd].astype(bf16)

        crb = np.zeros((128, CB), bf16)
        gi_np, cr_np = gidx[k]
        crb[:, 0:TT] = cr_np
        crb[:, C_IO:C_IO + 128] = iota.astype(bf16)
        crb[:, C_W2:C_W2 + 128] = w2cols.astype(bf16)
        crb[:, C_W3:C_W3 + 16] = w3cols.astype(bf16)
        crb[:, C_WK:C_WK + 128] = wkm.astype(bf16)
        crb[0, C_SD:C_SD + SH] = sqrtdeg[nd].astype(bf16)
        crb[0, C_SD + SDW:C_SD + SDW + 128] = b2.astype(bf16)
        crb[0, C_SD + SDW + 128:C_SD + SDW + 128 + C] = b3.astype(bf16)

        f32v = np.zeros((128, FB), np.float32)
        dloc = np.ones(BLK * 128, np.float32)
        dloc[:SH] = dinv[nd]
        f32v[:, 0:BLK] = dloc.reshape(BLK, 128).T
        d2loc = np.ones(BLK * 128, np.float32)
        d2loc[:SH] = dinv2[nd]
        f32v[:, F_D2:F_D2 + BLK] = d2loc.reshape(BLK, 128).T
        f32v[:, F_R:F_R + C * C] = np.tile(R.reshape(1, C * C), (128, 1))
        f32v[:, F_ID:F_ID + 128] = np.eye(128, dtype=np.float32)

        in_maps.append({"x128": x128, "gi": gi_np, "crb": crb, "f32b": f32v})

    trace = os.environ.get("GNN_TRACE") == "1"
    res = run_bass_kernel_spmd(nc, in_maps, core_ids=list(range(NCORES)),
                               trace=trace)
    LAST_RESULTS = res

    reps = int(os.environ.get("GNN_BENCH", "0"))
    if reps > 0:
        _bench(nc, in_maps, reps)
    return np.concatenate([res.results[k]["out"] for k in range(NCORES)],
                          axis=0)


BENCH_TIMES = None
BENCH_PIPELINED_NS = None


def _bench(nc, in_maps, reps):
    """Time repeated executions of the already-built program through a single
    jit instance (NEFF compile amortized away; inputs device_put once)."""
    global BENCH_TIMES
    import time
    import jax
    from jax.sharding import Mesh, PartitionSpec, NamedSharding
    from jax.experimental.shard_map import shard_map
    import concourse.mybir as mybir
    from concourse.bass2jax import (_bass_exec_p, partition_id_tensor,
                                    install_neuronx_cc_hook)

    install_neuronx_cc_hook()
    in_names, out_names, out_avals, zero_outs = [], [], [], []
    pname = nc.partition_id_tensor.name if nc.partition_id_tensor else None
    for alloc in nc.m.functions[0].allocations:
        if not isinstance(alloc, mybir.MemoryLocationSet):
            continue
        name = alloc.memorylocations[0].name
        if alloc.kind == "ExternalInput":
            if name != pname:
                in_names.append(name)
        elif alloc.kind == "ExternalOutput":
            out_names.append(name)
            shape = tuple(alloc.tensor_shape)
            dtype = mybir.dt.np(alloc.dtype)
            out_avals.append(jax.core.ShapedArray(shape, dtype))
            zero_outs.append(np.zeros(shape, dtype))
    n_params = len(in_names)
    all_names = in_names + out_names + ([pname] if pname else [])

    def _body(*args):
        ops = list(args)
        if pname:
            ops.append(partition_id_tensor())
        return tuple(_bass_exec_p.bind(
            *ops, out_avals=tuple(out_avals), in_names=tuple(all_names),
            out_names=tuple(out_names), lowering_input_output_aliases=(),
            sim_require_finite=True, sim_require_nnan=True, nc=nc))

    devices = jax.devices()[:NCORES]
    mesh = Mesh(np.asarray(devices), ("core",))
    nouts = len(out_names)
    sharded = jax.jit(
        shard_map(_body, mesh=mesh,
                  in_specs=(PartitionSpec("core"),) * (n_params + nouts),
                  out_specs=(PartitionSpec("core"),) * nouts, check_rep=False),
        donate_argnums=tuple(range(n_params, n_params + nouts)),
        keep_unused=True)
    sh = NamedSharding(mesh, PartitionSpec("core"))
    dev_in = [jax.device_put(
        np.concatenate([np.asarray(in_maps[c][nm]) for c in range(NCORES)],
                       axis=0), sh)
        for nm in in_names]
    jax.block_until_ready(dev_in)
    times = []
    for i in range(reps + 1):
        zs = [jax.device_put(
            np.zeros((NCORES * z.shape[0], *z.shape[1:]), z.dtype), sh)
            for z in zero_outs]
        t0 = time.perf_counter()
        outs = sharded(*dev_in, *zs)
        jax.block_until_ready(outs)
        times.append(time.perf_counter() - t0)
    BENCH_TIMES = times
    print("bench wall times (s):", " ".join(f"{t:.4f}" for t in times))
    print(f"bench min/median after warmup: {min(times[1:]):.4f} / "
          f"{sorted(times[1:])[len(times[1:]) // 2]:.4f}")

    # pipelined async dispatch: amortizes per-call RPC overhead
    NPIPE = 32
    best = None
    zss = [[jax.device_put(
        np.zeros((NCORES * z.shape[0], *z.shape[1:]), z.dtype), sh)
        for z in zero_outs] for _ in range(NPIPE)]
    for trial in range(2):
        jax.block_until_ready(zss)
        t0 = time.perf_counter()
        outs = None
        for i in range(NPIPE):
            outs = sharded(*dev_in, *zss[i])
        jax.block_until_ready(outs)
        tp = (time.perf_counter() - t0) / NPIPE
        best = tp if best is None or tp < best else best
        zss = [[jax.device_put(
            np.zeros((NCORES * z.shape[0], *z.shape[1:]), z.dtype), sh)
            for z in zero_outs] for _ in range(NPIPE)]
    global BENCH_PIPELINED_NS
    BENCH_PIPELINED_NS = int(best * 1e9)
    print(f"bench pipelined per-exec: {best * 1e3:.3f} ms "
          f"({best * 1e9:.0f} ns upper bound)")


# revision 3
# speedup vs baseline: 1.1344x; 1.1137x over previous
"""Bass/Trainium2 kernel for HCFC-GNN (3-layer GCN + hierarchy max-constraint).

Strategy (8 NeuronCores, SPMD, pull-mode with target-sharded edges):
  - Nodes sharded 6250/core; edges (incl. self-loops) sharded by TARGET core,
    sorted by (target block, source half). Self loops ride as plain edges.
  - Layer-1 aggregation operates on the PRE-transform features: the table is
    t1 = dinv * [x | 1] (host-built, bf16, 16 cols in 128-col rows), so the
    L1 dense transform folds in AFTER aggregation via Wk = [W1^T; b1]. This
    kills one on-device transform pass + one AllGather.
  - GCN norm folding: raw agg A[c] = sum_{e->c} t[src]; per node n the next
    table row is t' = dinv_n^2 * (relu(A_n) @ W^T) + dinv_n * b, realized as
    matmul + rank-1 bias (sqrtdeg_n outer b) + one per-partition scale.
  - Scatter-add via PE one-hot matmul. L1/L2 run "swapped" (stationary=msg,
    streamed=S) so the accumulator lands feature-major, which is exactly the
    lhsT the following dense transform wants -> no per-block PE transpose.
    L3 runs un-swapped so the final [node,16] lands node-major for
    sigmoid + R-max directly.
  - Gathers read straight from the AllGather output (no private copy) and
    round-robin across 4 SWDGE queues.
  - Inputs packed into 4 tensors (x128 table shard, int16 gather indices,
    bf16 blob, f32 blob) -- per-exec dispatch overhead scales with the
    input-tensor list, not with device work.
"""

import os
import numpy as np
import ml_dtypes

N = 50000
E = 1600000
C = 13
DIN = 12
H = 128
NCORES = 8
SH = N // NCORES          # 6250 nodes per shard
CH = 6272                 # shard chunk rows in gathered table (6250 + 22 pad)
BLK = (SH + 127) // 128   # 49 blocks per shard (last block 106 nodes)
LASTB = SH - (BLK - 1) * 128  # 106
HALF = 4 * CH             # 25088 rows per gather half (int16-safe)
ZROW = SH                 # pad row index inside each half
PADCREL = 300.0           # colrel value guaranteed not to match iota 0..127

bf16 = ml_dtypes.bfloat16

LAST_RESULTS = None


def _prep_edges(edge_index):
    """Partition/sort edges; build per-core merged gather-index and colrel
    streams with slot sizes (TL) uniform across cores so one SPMD program
    works. Returns deg, TL, off (abs tile offsets incl. half concat), TOT
    (total tiles lo/hi), and per-core (gi, cr) arrays."""
    row = np.concatenate([edge_index[0], np.arange(N, dtype=np.int32)])
    col = np.concatenate([edge_index[1], np.arange(N, dtype=np.int32)])
    deg = np.bincount(row, minlength=N).astype(np.float32)

    s_shard = row // SH
    grow = s_shard * CH + (row % SH)       # row index in gathered table
    half = (grow >= HALF).astype(np.int64)
    gloc = np.where(half == 0, grow, grow - HALF).astype(np.int64)
    tcore = col // SH
    tcol = col % SH
    blk = tcol // 128
    crel = (tcol % 128).astype(np.int64)

    key = ((tcore * BLK) + blk) * 2 + half
    order = np.lexsort((gloc, key))
    key_s = key[order]
    gloc_s = gloc[order]
    crel_s = crel[order]

    nslots = NCORES * BLK * 2
    cnt = np.bincount(key_s, minlength=nslots).reshape(NCORES, BLK, 2)
    starts = np.zeros(nslots + 1, np.int64)
    np.cumsum(cnt.reshape(-1), out=starts[1:])

    # uniform tile counts across cores
    TL = np.maximum(1, ((cnt + 127) // 128).max(axis=0))  # [BLK, 2]
    TOT = [int(TL[:, 0].sum()), int(TL[:, 1].sum())]
    off = np.zeros((BLK, 2), np.int64)   # absolute tile offset in merged gi
    tot = [0, 0]
    for hh in (0, 1):
        for b in range(BLK):
            off[b, hh] = tot[hh] + (TOT[0] if hh == 1 else 0)
            tot[hh] += TL[b, hh]

    gidx = []   # per core: (gi [128, 8*TT] int16, cr [128, TT] bf16)
    for k in range(NCORES):
        gparts, cparts = [], []
        for hh in (0, 1):
            for b in range(BLK):
                s = starts[(k * BLK + b) * 2 + hh]
                e = starts[(k * BLK + b) * 2 + hh + 1]
                n = int(e - s)
                m = int(TL[b, hh]) * 128
                gseg = np.full(m, ZROW, np.int64)
                cseg = np.full(m, PADCREL, np.float64)
                gseg[:n] = gloc_s[s:e]
                cseg[:n] = crel_s[s:e]
                # wrapped idx layout: pos i -> partition i%16, col i//16
                gparts.append(gseg.reshape(m // 16, 16).T.astype(np.int16))
                # colrel layout: pos i -> partition i%128, col i//128
                cparts.append(cseg.reshape(m // 128, 128).T.astype(bf16))
        g = np.hstack(gparts)                      # [16, TT*8]
        gi = np.ascontiguousarray(np.tile(g, (8, 1)))   # [128, TT*8]
        cr = np.ascontiguousarray(np.hstack(cparts))    # [128, TT]
        gidx.append((gi, cr))
    return deg, TL, off, TOT, gidx


def _build_program(TL, off, TOT):
    import concourse.bacc as bacc
    import concourse.mybir as mybir
    import concourse.tile as tile

    dt = mybir.dt
    nc = bacc.Bacc("TRN2", target_bir_lowering=False, debug=False,
                   num_devices=NCORES, num_swdge_queues=4)

    TT = TOT[0] + TOT[1]
    TLMAX = int(TL.max())
    SDW = BLK * 128          # 6272 cols for sqrtdeg row
    # bf16 blob layout (cols): cr [0:TT], iota, W2cols, W3cols, Wk, sd-region
    C_IO = TT
    C_W2 = C_IO + 128
    C_W3 = C_W2 + 128
    C_WK = C_W3 + 16
    C_SD = C_WK + 128
    CB = C_SD + SDW + 144
    # f32 blob layout: dinv [0:BLK], dinv2, Rfl(169), ident(128)
    F_D2 = BLK
    F_R = 2 * BLK
    F_ID = F_R + C * C
    FB = F_ID + 128

    x128 = nc.dram_tensor("x128", [CH, H], dt.bfloat16, kind="ExternalInput")
    gi = nc.dram_tensor("gi", [128, 8 * TT], dt.int16, kind="ExternalInput")
    crb = nc.dram_tensor("crb", [128, CB], dt.bfloat16, kind="ExternalInput")
    f32b = nc.dram_tensor("f32b", [128, FB], dt.float32, kind="ExternalInput")
    out = nc.dram_tensor("out", [SH, C], dt.float32, kind="ExternalOutput")

    gin = nc.dram_tensor("gin", [CH, H], dt.bfloat16)
    gout_a = nc.dram_tensor("gout_a", [NCORES * CH, H], dt.bfloat16,
                            addr_space="Shared")
    gout_b = nc.dram_tensor("gout_b", [NCORES * CH, H], dt.bfloat16,
                            addr_space="Shared")

    qn = [0]  # gather queue rotation

    with tile.TileContext(nc) as tc:
        with (
            tc.tile_pool(name="const", bufs=1) as cpool,
            tc.tile_pool(name="idx", bufs=1) as ipool,
            tc.tile_pool(name="msg", bufs=6) as mpool,
            tc.tile_pool(name="sbl", bufs=6) as spool,
            tc.tile_pool(name="hblk", bufs=4) as hpool,
            tc.tile_pool(name="gblk", bufs=3) as gpool,
            tc.tile_pool(name="psum", bufs=3, space="PSUM") as pp,
            tc.tile_pool(name="psumt", bufs=3, space="PSUM") as ppt,
        ):
            # ---- loads ----
            gi_t = ipool.tile([128, 8 * TT], dt.int16)
            nc.sync.dma_start(out=gi_t[:], in_=gi[:])
            cr_t = cpool.tile([128, TT], dt.bfloat16)
            nc.sync.dma_start(out=cr_t[:], in_=crb[:, 0:TT])
            io_t = cpool.tile([128, 128], dt.bfloat16)
            nc.sync.dma_start(out=io_t[:], in_=crb[:, C_IO:C_IO + 128])
            w2c = cpool.tile([128, 128], dt.bfloat16)
            nc.sync.dma_start(out=w2c[:], in_=crb[:, C_W2:C_W2 + 128])
            w3c = cpool.tile([128, 16], dt.bfloat16)
            nc.sync.dma_start(out=w3c[:], in_=crb[:, C_W3:C_W3 + 16])
            wk = cpool.tile([128, 128], dt.bfloat16)
            nc.sync.dma_start(out=wk[:], in_=crb[:, C_WK:C_WK + 128])
            sdb = cpool.tile([128, SDW + 144], dt.bfloat16)
            nc.sync.dma_start(out=sdb[:], in_=crb[:, C_SD:C_SD + SDW + 144])
            dinv_t = cpool.tile([128, BLK], dt.float32)
            nc.sync.dma_start(out=dinv_t[:], in_=f32b[:, 0:BLK])
            dinv2_t = cpool.tile([128, BLK], dt.float32)
            nc.sync.dma_start(out=dinv2_t[:], in_=f32b[:, F_D2:F_D2 + BLK])
            r_t = cpool.tile([128, C * C], dt.float32)
            nc.sync.dma_start(out=r_t[:], in_=f32b[:, F_R:F_R + C * C])
            id_t = cpool.tile([128, 128], dt.float32)
            nc.sync.dma_start(out=id_t[:], in_=f32b[:, F_ID:F_ID + 128])

            # L1 table shard -> gin, AllGather
            nc.sync.dma_start(out=gin[:, :], in_=x128[:, :])
            nc.gpsimd.collective_compute(
                "AllGather", mybir.AluOpType.bypass,
                replica_groups=[list(range(NCORES))],
                ins=[gin[:, :]], outs=[gout[:, :]],
            )

            halves = (gout[0:HALF, :], gout[HALF:2 * HALF, :])

            def agg_block(b, width, swapped):
                """Gather + scatter for node block b.
                swapped: acc[0:width,0:128] = msg^T @ S (feature-major).
                else:    acc[0:128,0:width] = S^T @ msg (node-major)."""
                acc = pp.tile([128, 128], dt.float32, tag="aggpsum")
                first = True
                for hh in (0, 1):
                    tl = int(TL[b, hh])
                    o = int(off[b, hh])
                    msg = mpool.tile([128, TLMAX, H], dt.bfloat16, tag="msg")
                    nc.gpsimd.dma_gather(
                        out_ap=msg[:, 0:tl, :], in_ap=halves[hh],
                        idxs_ap=gi_t[:, o * 8:(o + tl) * 8],
                        num_idxs=tl * 128, num_idxs_reg=tl * 128, elem_size=H,
                        single_packet=False, queue_num=qn[0],
                    )
                    qn[0] = (qn[0] + 1) % 4
                    S = spool.tile([128, TLMAX, 128], dt.bfloat16, tag="sb")
                    nc.vector.tensor_tensor(
                        out=S[:, 0:tl, :],
                        in0=cr_t[:, o:o + tl].unsqueeze(2)
                            .broadcast_to([128, tl, 128]),
                        in1=io_t[:, :].unsqueeze(1).broadcast_to([128, tl, 128]),
                        op=mybir.AluOpType.is_equal,
                    )
                    for j in range(tl):
                        last = (hh == 1 and j == int(TL[b, 1]) - 1)
                        if swapped:
                            nc.tensor.matmul(acc[0:width, :],
                                             msg[:, j, 0:width], S[:, j, :],
                                             start=first, stop=last)
                        else:
                            nc.tensor.matmul(acc[:, 0:width],
                                             S[:, j, :], msg[:, j, 0:width],
                                             start=first, stop=last)
                        first = False
                return acc

            # ---------------- Layer 1: agg(t1) -> h1 -> t2 ------------------
            for b in range(BLK):
                acc = agg_block(b, 16, swapped=True)
                a1 = hpool.tile([16, 128], dt.bfloat16, tag="a1")
                nc.vector.tensor_copy(a1[:, :], acc[0:16, :])
                h1 = ppt.tile([128, 128], dt.float32, tag="tp")
                nc.tensor.matmul(h1[:, :], a1[:, :], wk[0:16, :],
                                 start=True, stop=True)
                p1 = hpool.tile([128, 128], dt.float32, tag="p1")
                nc.scalar.activation(p1[:, :], h1[:, :],
                                     mybir.ActivationFunctionType.Relu)
                tp = ppt.tile([128, 128], dt.float32, tag="tp")
                nc.tensor.transpose(tp[:, :], p1[:, :],
                                    io_t[:, :])  # identity via f32? see below
                p1b = hpool.tile([128, 128], dt.bfloat16, tag="p1b")
                nc.vector.tensor_copy(p1b[:, :], tp[:, :])
                acc2 = ppt.tile([128, 128], dt.float32, tag="tp")
                nc.tensor.matmul(acc2[:, :], p1b[:, :], w2c[:, :],
                                 start=True, stop=False)
                nc.tensor.matmul(acc2[:, :], sdb[0:1, b * 128:b * 128 + 128],
                                 sdb[0:1, SDW:SDW + 128], start=False,
                                 stop=True)
                g2 = gpool.tile([128, 128], dt.bfloat16, tag="g")
                nc.vector.tensor_scalar_mul(g2[:, :], acc2[:, :],
                                            dinv2_t[:, b:b + 1])
                nc.sync.dma_start(out=gin[b * 128:b * 128 + 128, :],
                                  in_=g2[:, :])
            nc.gpsimd.collective_compute(
                "AllGather", mybir.AluOpType.bypass,
                replica_groups=[list(range(NCORES))],
                ins=[gin[:, :]], outs=[gout[:, :]],
            )

            # ---------------- Layer 2: agg(t2) -> h2 -> t3 ------------------
            for b in range(BLK):
                acc = agg_block(b, 128, swapped=True)
                p2 = hpool.tile([128, 128], dt.bfloat16, tag="p1b")
                nc.scalar.activation(p2[:, :], acc[:, :],
                                     mybir.ActivationFunctionType.Relu)
                acc3 = ppt.tile([128, 128], dt.float32, tag="tp")
                nc.tensor.matmul(acc3[:, 0:16], p2[:, :], w3c[:, :],
                                 start=True, stop=False)
                nc.tensor.matmul(acc3[:, 0:16],
                                 sdb[0:1, b * 128:b * 128 + 128],
                                 sdb[0:1, SDW + 128:SDW + 144], start=False,
                                 stop=True)
                g3 = gpool.tile([128, 16], dt.bfloat16, tag="g3")
                nc.vector.tensor_scalar_mul(g3[:, :], acc3[:, 0:16],
                                            dinv2_t[:, b:b + 1])
                nc.sync.dma_start(out=gin[b * 128:b * 128 + 128, 0:16],
                                  in_=g3[:, :])
            nc.gpsimd.collective_compute(
                "AllGather", mybir.AluOpType.bypass,
                replica_groups=[list(range(NCORES))],
                ins=[gin[:, :]], outs=[gout[:, :]],
            )

            # ---------------- Layer 3: agg(t3) -> sigmoid -> R-max ----------
            for b in range(BLK):
                acc = agg_block(b, 16, swapped=False)
                sg = hpool.tile([128, 16], dt.float32, tag="sg")
                nc.scalar.activation(sg[:, :], acc[:, 0:16],
                                     mybir.ActivationFunctionType.Sigmoid,
                                     scale=dinv_t[:, b:b + 1])
                tmp = hpool.tile([128, C, C], dt.float32, tag="tmp")
                nc.vector.tensor_tensor(
                    out=tmp[:, :, :],
                    in0=sg[:, 0:C].unsqueeze(1).broadcast_to([128, C, C]),
                    in1=r_t[:, :].rearrange("p (a b) -> p a b", a=C),
                    op=mybir.AluOpType.mult,
                )
                o13 = gpool.tile([128, C], dt.float32, tag="o13")
                nc.vector.tensor_reduce(o13[:, :], tmp[:, :, :],
                                        axis=mybir.AxisListType.X,
                                        op=mybir.AluOpType.max)
                rows = 128 if b < BLK - 1 else LASTB
                nc.sync.dma_start(out=out[b * 128:b * 128 + rows, :],
                                  in_=o13[0:rows, :])

    nc.compile()
    return nc


def kernel(x, edge_index, R, W1, b1, W2, b2, W3, b3, **_):
    global LAST_RESULTS
    import concourse.mybir  # noqa: F401
    from concourse.bass_utils import run_bass_kernel_spmd

    x = np.asarray(x, np.float32)
    edge_index = np.asarray(edge_index, np.int32)
    deg, TL, off, TOT, gidx = _prep_edges(edge_index)

    nc = _build_program(TL, off, TOT)

    TT = TOT[0] + TOT[1]
    SDW = BLK * 128
    C_IO = TT
    C_W2 = C_IO + 128
    C_W3 = C_W2 + 128
    C_WK = C_W3 + 16
    C_SD = C_WK + 128
    CB = C_SD + SDW + 144
    F_D2 = BLK
    F_R = 2 * BLK
    F_ID = F_R + C * C
    FB = F_ID + 128

    dinv = 1.0 / np.sqrt(deg)
    dinv2 = dinv * dinv
    sqrtdeg = np.sqrt(deg)

    W1 = np.asarray(W1, np.float32)
    W2 = np.asarray(W2, np.float32)
    W3 = np.asarray(W3, np.float32)
    b1 = np.asarray(b1, np.float32)
    b2 = np.asarray(b2, np.float32)
    b3 = np.asarray(b3, np.float32)
    R = np.asarray(R, np.float32)

    # shared bf16 blob pieces
    iota = np.tile(np.arange(128, dtype=np.float32), (128, 1))
    w2cols = W2.T                                   # [H(in f), H(out o)]
    w3cols = np.zeros((128, 16), np.float32)
    w3cols[:, :C] = W3.T
    wkm = np.zeros((128, 128), np.float32)
    wkm[0:DIN, :] = W1.T
    wkm[DIN, :] = b1

    in_maps = []
    for k in range(NCORES):
        nd = slice(k * SH, (k + 1) * SH)
        x128 = np.zeros((CH, H), bf16)
        x128[:SH, 0:DIN] = (x[nd] * dinv[nd][:, None]).astype(bf16)
        x128[:SH, DIN] = dinv[nd].astype(bf16)

        crb = np.zeros((128, CB), bf16)
        gi_np, cr_np = gidx[k]
        crb[:, 0:TT] = cr_np
        crb[:, C_IO:C_IO + 128] = iota.astype(bf16)
        crb[:, C_W2:C_W2 + 128] = w2cols.astype(bf16)
        crb[:, C_W3:C_W3 + 16] = w3cols.astype(bf16)
        crb[:, C_WK:C_WK + 128] = wkm.astype(bf16)
        crb[0, C_SD:C_SD + SH] = sqrtdeg[nd].astype(bf16)
        crb[0, C_SD + SDW:C_SD + SDW + 128] = b2.astype(bf16)
        crb[0, C_SD + SDW + 128:C_SD + SDW + 128 + C] = b3.astype(bf16)

        f32v = np.zeros((128, FB), np.float32)
        dloc = np.ones(BLK * 128, np.float32)
        dloc[:SH] = dinv[nd]
        f32v[:, 0:BLK] = dloc.reshape(BLK, 128).T
        d2loc = np.ones(BLK * 128, np.float32)
        d2loc[:SH] = dinv2[nd]
        f32v[:, F_D2:F_D2 + BLK] = d2loc.reshape(BLK, 128).T
        f32v[:, F_R:F_R + C * C] = np.tile(R.reshape(1, C * C), (128, 1))
        f32v[:, F_ID:F_ID + 128] = np.eye(128, dtype=np.float32)

        in_maps.append({"x128": x128, "gi": gi_np, "crb": crb, "f32b": f32v})

    trace = os.environ.get("GNN_TRACE") == "1"
    res = run_bass_kernel_spmd(nc, in_maps, core_ids=list(range(NCORES)),
                               trace=trace)
    LAST_RESULTS = res

    reps = int(os.environ.get("GNN_BENCH", "0"))
    if reps > 0:
        _bench(nc, in_maps, reps)
    return np.concatenate([res.results[k]["out"] for k in range(NCORES)],
                          axis=0)


BENCH_TIMES = None
BENCH_PIPELINED_NS = None


def _bench(nc, in_maps, reps):
    """Time repeated executions of the already-built program through a single
    jit instance (NEFF compile amortized away; inputs device_put once)."""
    global BENCH_TIMES
    import time
    import jax
    from jax.sharding import Mesh, PartitionSpec, NamedSharding
    from jax.experimental.shard_map import shard_map
    import concourse.mybir as mybir
    from concourse.bass2jax import (_bass_exec_p, partition_id_tensor,
                                    install_neuronx_cc_hook)

    install_neuronx_cc_hook()
    in_names, out_names, out_avals, zero_outs = [], [], [], []
    pname = nc.partition_id_tensor.name if nc.partition_id_tensor else None
    for alloc in nc.m.functions[0].allocations:
        if not isinstance(alloc, mybir.MemoryLocationSet):
            continue
        name = alloc.memorylocations[0].name
        if alloc.kind == "ExternalInput":
            if name != pname:
                in_names.append(name)
        elif alloc.kind == "ExternalOutput":
            out_names.append(name)
            shape = tuple(alloc.tensor_shape)
            dtype = mybir.dt.np(alloc.dtype)
            out_avals.append(jax.core.ShapedArray(shape, dtype))
            zero_outs.append(np.zeros(shape, dtype))
    n_params = len(in_names)
    all_names = in_names + out_names + ([pname] if pname else [])

    def _body(*args):
        ops = list(args)
        if pname:
            ops.append(partition_id_tensor())
        return tuple(_bass_exec_p.bind(
            *ops, out_avals=tuple(out_avals), in_names=tuple(all_names),
            out_names=tuple(out_names), lowering_input_output_aliases=(),
            sim_require_finite=True, sim_require_nnan=True, nc=nc))

    devices = jax.devices()[:NCORES]
    mesh = Mesh(np.asarray(devices), ("core",))
    nouts = len(out_names)
    sharded = jax.jit(
        shard_map(_body, mesh=mesh,
                  in_specs=(PartitionSpec("core"),) * (n_params + nouts),
                  out_specs=(PartitionSpec("core"),) * nouts, check_rep=False),
        donate_argnums=tuple(range(n_params, n_params + nouts)),
        keep_unused=True)
    sh = NamedSharding(mesh, PartitionSpec("core"))
    dev_in = [jax.device_put(
        np.concatenate([np.asarray(in_maps[c][nm]) for c in range(NCORES)],
                       axis=0), sh)
        for nm in in_names]
    jax.block_until_ready(dev_in)
    times = []
    for i in range(reps + 1):
        zs = [jax.device_put(
            np.zeros((NCORES * z.shape[0], *z.shape[1:]), z.dtype), sh)
            for z in zero_outs]
        t0 = time.perf_counter()
        outs = sharded(*dev_in, *zs)
        jax.block_until_ready(outs)
        times.append(time.perf_counter() - t0)
    BENCH_TIMES = times
    print("bench wall times (s):", " ".join(f"{t:.4f}" for t in times))
    print(f"bench min/median after warmup: {min(times[1:]):.4f} / "
          f"{sorted(times[1:])[len(times[1:]) // 2]:.4f}")

    # pipelined async dispatch: amortizes per-call RPC overhead
    NPIPE = 48
    best = None
    zss = [[jax.device_put(
        np.zeros((NCORES * z.shape[0], *z.shape[1:]), z.dtype), sh)
        for z in zero_outs] for _ in range(NPIPE)]
    for trial in range(3):
        jax.block_until_ready(zss)
        t0 = time.perf_counter()
        outs = None
        for i in range(NPIPE):
            outs = sharded(*dev_in, *zss[i])
        jax.block_until_ready(outs)
        tp = (time.perf_counter() - t0) / NPIPE
        best = tp if best is None or tp < best else best
        zss = [[jax.device_put(
            np.zeros((NCORES * z.shape[0], *z.shape[1:]), z.dtype), sh)
            for z in zero_outs] for _ in range(NPIPE)]
    global BENCH_PIPELINED_NS
    BENCH_PIPELINED_NS = int(best * 1e9)
    print(f"bench pipelined per-exec: {best * 1e3:.3f} ms "
          f"({best * 1e9:.0f} ns upper bound)")


# revision 5
# speedup vs baseline: 1.5172x; 1.3375x over previous
"""Bass/Trainium2 kernel for HCFC-GNN (3-layer GCN + hierarchy max-constraint).

Strategy (8 NeuronCores, SPMD, pull-mode with target-sharded edges):
  - Nodes sharded 6250/core; edges (incl. self-loops) sharded by TARGET core,
    sorted by (target block, source half). Self loops ride as plain edges.
  - Layer-1 aggregation operates on the PRE-transform features: the table is
    t1 = dinv * [x | 1] (host-built, bf16, 16 cols in 128-col rows), so the
    L1 dense transform folds in AFTER aggregation via Wk = [W1^T; b1]. This
    kills one on-device transform pass + one AllGather.
  - GCN norm folding: raw agg A[c] = sum_{e->c} t[src]; per node n the next
    table row is t' = dinv_n^2 * (relu(A_n) @ W^T) + dinv_n * b, realized as
    matmul + rank-1 bias (sqrtdeg_n outer b) + one per-partition scale.
  - Scatter-add via PE one-hot matmul. L1/L2 run "swapped" (stationary=msg,
    streamed=S) so the accumulator lands feature-major, which is exactly the
    lhsT the following dense transform wants -> no per-block PE transpose.
    L3 runs un-swapped so the final [node,16] lands node-major for
    sigmoid + R-max directly.
  - Gathers read straight from the AllGather output (no private copy) and
    round-robin across 4 SWDGE queues.
  - Inputs packed into 4 tensors (x128 table shard, int16 gather indices,
    bf16 blob, f32 blob) -- per-exec dispatch overhead scales with the
    input-tensor list, not with device work.
"""

import os
import numpy as np
import ml_dtypes

N = 50000
E = 1600000
C = 13
DIN = 12
H = 128
NCORES = 8
SH = N // NCORES          # 6250 nodes per shard
CH = 6272                 # shard chunk rows in gathered table (6250 + 22 pad)
BLK = (SH + 127) // 128   # 49 blocks per shard (last block 106 nodes)
LASTB = SH - (BLK - 1) * 128  # 106
HALF = 4 * CH             # 25088 rows per gather half (int16-safe)
ZROW = SH                 # pad row index inside each half
PADCREL = 300.0           # colrel value guaranteed not to match iota 0..127

bf16 = ml_dtypes.bfloat16

LAST_RESULTS = None


def _prep_edges(edge_index):
    """Partition/sort edges; build per-core merged gather-index and colrel
    streams with slot sizes (TL) uniform across cores so one SPMD program
    works. Returns deg, TL, off (abs tile offsets incl. half concat), TOT
    (total tiles lo/hi), and per-core (gi, cr) arrays."""
    row = np.concatenate([edge_index[0], np.arange(N, dtype=np.int32)])
    col = np.concatenate([edge_index[1], np.arange(N, dtype=np.int32)])
    deg = np.bincount(row, minlength=N).astype(np.float32)

    s_shard = row // SH
    grow = s_shard * CH + (row % SH)       # row index in gathered table
    half = (grow >= HALF).astype(np.int64)
    gloc = np.where(half == 0, grow, grow - HALF).astype(np.int64)
    tcore = col // SH
    tcol = col % SH
    blk = tcol // 128
    crel = (tcol % 128).astype(np.int64)

    key = ((tcore * BLK) + blk) * 2 + half
    order = np.lexsort((gloc, key))
    key_s = key[order]
    gloc_s = gloc[order]
    crel_s = crel[order]

    nslots = NCORES * BLK * 2
    cnt = np.bincount(key_s, minlength=nslots).reshape(NCORES, BLK, 2)
    starts = np.zeros(nslots + 1, np.int64)
    np.cumsum(cnt.reshape(-1), out=starts[1:])

    # uniform tile counts across cores
    TL = np.maximum(1, ((cnt + 127) // 128).max(axis=0))  # [BLK, 2]
    TOT = [int(TL[:, 0].sum()), int(TL[:, 1].sum())]
    off = np.zeros((BLK, 2), np.int64)   # absolute tile offset in merged gi
    tot = [0, 0]
    for hh in (0, 1):
        for b in range(BLK):
            off[b, hh] = tot[hh] + (TOT[0] if hh == 1 else 0)
            tot[hh] += TL[b, hh]

    gidx = []   # per core: (gi [128, 8*TT] int16, cr [128, TT] bf16)
    for k in range(NCORES):
        gparts, cparts = [], []
        for hh in (0, 1):
            for b in range(BLK):
                s = starts[(k * BLK + b) * 2 + hh]
                e = starts[(k * BLK + b) * 2 + hh + 1]
                n = int(e - s)
                m = int(TL[b, hh]) * 128
                gseg = np.full(m, ZROW, np.int64)
                cseg = np.full(m, PADCREL, np.float64)
                gseg[:n] = gloc_s[s:e]
                cseg[:n] = crel_s[s:e]
                # wrapped idx layout: pos i -> partition i%16, col i//16
                gparts.append(gseg.reshape(m // 16, 16).T.astype(np.int16))
                # colrel layout: pos i -> partition i%128, col i//128
                cparts.append(cseg.reshape(m // 128, 128).T.astype(bf16))
        g = np.hstack(gparts)                      # [16, TT*8]
        gi = np.ascontiguousarray(np.tile(g, (8, 1)))   # [128, TT*8]
        cr = np.ascontiguousarray(np.hstack(cparts))    # [128, TT]
        gidx.append((gi, cr))
    return deg, TL, off, TOT, gidx


def _build_program(TL, off, TOT):
    import concourse.bacc as bacc
    import concourse.mybir as mybir
    import concourse.tile as tile

    dt = mybir.dt
    nc = bacc.Bacc("TRN2", target_bir_lowering=False, debug=False,
                   num_devices=NCORES, num_swdge_queues=4)

    TT = TOT[0] + TOT[1]
    TLMAX = int(TL.max())
    SDW = BLK * 128          # 6272 cols for sqrtdeg row
    # bf16 blob layout (cols): cr [0:TT], iota, W2cols, W3cols, Wk, sd-region
    C_IO = TT
    C_W2 = C_IO + 128
    C_W3 = C_W2 + 128
    C_WK = C_W3 + 16
    C_SD = C_WK + 128
    CB = C_SD + SDW + 144
    # f32 blob layout: dinv [0:BLK], dinv2, Rfl(169), ident(128)
    F_D2 = BLK
    F_R = 2 * BLK
    F_ID = F_R + C * C
    FB = F_ID + 128

    x128 = nc.dram_tensor("x128", [CH, H], dt.bfloat16, kind="ExternalInput")
    gi = nc.dram_tensor("gi", [128, 8 * TT], dt.int16, kind="ExternalInput")
    crb = nc.dram_tensor("crb", [128, CB], dt.bfloat16, kind="ExternalInput")
    f32b = nc.dram_tensor("f32b", [128, FB], dt.float32, kind="ExternalInput")
    out = nc.dram_tensor("out", [SH, C], dt.float32, kind="ExternalOutput")

    gin = nc.dram_tensor("gin", [CH, H], dt.bfloat16)
    gout_a = nc.dram_tensor("gout_a", [NCORES * CH, H], dt.bfloat16,
                            addr_space="Shared")
    gout_b = nc.dram_tensor("gout_b", [NCORES * CH, H], dt.bfloat16,
                            addr_space="Shared")

    qn = [0]  # gather queue rotation

    with tile.TileContext(nc) as tc:
        with (
            tc.tile_pool(name="const", bufs=1) as cpool,
            tc.tile_pool(name="idx", bufs=1) as ipool,
            tc.tile_pool(name="msg", bufs=6) as mpool,
            tc.tile_pool(name="sbl", bufs=6) as spool,
            tc.tile_pool(name="hblk", bufs=4) as hpool,
            tc.tile_pool(name="gblk", bufs=3) as gpool,
            tc.tile_pool(name="psum", bufs=3, space="PSUM") as pp,
            tc.tile_pool(name="psumt", bufs=3, space="PSUM") as ppt,
        ):
            # ---- loads ----
            gi_t = ipool.tile([128, 8 * TT], dt.int16)
            nc.sync.dma_start(out=gi_t[:], in_=gi[:])
            cr_t = cpool.tile([128, TT], dt.bfloat16)
            nc.sync.dma_start(out=cr_t[:], in_=crb[:, 0:TT])
            io_t = cpool.tile([128, 128], dt.bfloat16)
            nc.sync.dma_start(out=io_t[:], in_=crb[:, C_IO:C_IO + 128])
            w2c = cpool.tile([128, 128], dt.bfloat16)
            nc.sync.dma_start(out=w2c[:], in_=crb[:, C_W2:C_W2 + 128])
            w3c = cpool.tile([128, 16], dt.bfloat16)
            nc.sync.dma_start(out=w3c[:], in_=crb[:, C_W3:C_W3 + 16])
            wk = cpool.tile([128, 128], dt.bfloat16)
            nc.sync.dma_start(out=wk[:], in_=crb[:, C_WK:C_WK + 128])
            sdb = cpool.tile([128, SDW + 144], dt.bfloat16)
            nc.sync.dma_start(out=sdb[:], in_=crb[:, C_SD:C_SD + SDW + 144])
            dinv_t = cpool.tile([128, BLK], dt.float32)
            nc.sync.dma_start(out=dinv_t[:], in_=f32b[:, 0:BLK])
            dinv2_t = cpool.tile([128, BLK], dt.float32)
            nc.sync.dma_start(out=dinv2_t[:], in_=f32b[:, F_D2:F_D2 + BLK])
            r_t = cpool.tile([128, C * C], dt.float32)
            nc.sync.dma_start(out=r_t[:], in_=f32b[:, F_R:F_R + C * C])
            id_t = cpool.tile([128, 128], dt.float32)
            nc.sync.dma_start(out=id_t[:], in_=f32b[:, F_ID:F_ID + 128])

            # L1 table shard -> gin, AllGather
            nc.sync.dma_start(out=gin[:, :], in_=x128[:, :])
            nc.gpsimd.collective_compute(
                "AllGather", mybir.AluOpType.bypass,
                replica_groups=[list(range(NCORES))],
                ins=[gin[:, :]], outs=[gout[:, :]],
            )

            halves = (gout[0:HALF, :], gout[HALF:2 * HALF, :])

            def agg_block(b, width, swapped):
                """Gather + scatter for node block b.
                swapped: acc[0:width,0:128] = msg^T @ S (feature-major).
                else:    acc[0:128,0:width] = S^T @ msg (node-major)."""
                acc = pp.tile([128, 128], dt.float32, tag="aggpsum")
                first = True
                for hh in (0, 1):
                    tl = int(TL[b, hh])
                    o = int(off[b, hh])
                    msg = mpool.tile([128, TLMAX, H], dt.bfloat16, tag="msg")
                    nc.gpsimd.dma_gather(
                        out_ap=msg[:, 0:tl, :], in_ap=halves[hh],
                        idxs_ap=gi_t[:, o * 8:(o + tl) * 8],
                        num_idxs=tl * 128, num_idxs_reg=tl * 128, elem_size=H,
                        single_packet=False, queue_num=qn[0],
                    )
                    qn[0] = (qn[0] + 1) % 4
                    S = spool.tile([128, TLMAX, 128], dt.bfloat16, tag="sb")
                    nc.vector.tensor_tensor(
                        out=S[:, 0:tl, :],
                        in0=cr_t[:, o:o + tl].unsqueeze(2)
                            .broadcast_to([128, tl, 128]),
                        in1=io_t[:, :].unsqueeze(1).broadcast_to([128, tl, 128]),
                        op=mybir.AluOpType.is_equal,
                    )
                    for j in range(tl):
                        last = (hh == 1 and j == int(TL[b, 1]) - 1)
                        if swapped:
                            nc.tensor.matmul(acc[0:width, :],
                                             msg[:, j, 0:width], S[:, j, :],
                                             start=first, stop=last)
                        else:
                            nc.tensor.matmul(acc[:, 0:width],
                                             S[:, j, :], msg[:, j, 0:width],
                                             start=first, stop=last)
                        first = False
                return acc

            # ---------------- Layer 1: agg(t1) -> h1 -> t2 ------------------
            for b in range(BLK):
                acc = agg_block(b, 16, swapped=True)
                a1 = hpool.tile([16, 128], dt.bfloat16, tag="a1")
                nc.vector.tensor_copy(a1[:, :], acc[0:16, :])
                h1 = ppt.tile([128, 128], dt.float32, tag="tp")
                nc.tensor.matmul(h1[:, :], a1[:, :], wk[0:16, :],
                                 start=True, stop=True)
                p1 = hpool.tile([128, 128], dt.float32, tag="p1")
                nc.scalar.activation(p1[:, :], h1[:, :],
                                     mybir.ActivationFunctionType.Relu)
                tp = ppt.tile([128, 128], dt.float32, tag="tp")
                nc.tensor.transpose(tp[:, :], p1[:, :],
                                    io_t[:, :])  # identity via f32? see below
                p1b = hpool.tile([128, 128], dt.bfloat16, tag="p1b")
                nc.vector.tensor_copy(p1b[:, :], tp[:, :])
                acc2 = ppt.tile([128, 128], dt.float32, tag="tp")
                nc.tensor.matmul(acc2[:, :], p1b[:, :], w2c[:, :],
                                 start=True, stop=False)
                nc.tensor.matmul(acc2[:, :], sdb[0:1, b * 128:b * 128 + 128],
                                 sdb[0:1, SDW:SDW + 128], start=False,
                                 stop=True)
                g2 = gpool.tile([128, 128], dt.bfloat16, tag="g")
                nc.vector.tensor_scalar_mul(g2[:, :], acc2[:, :],
                                            dinv2_t[:, b:b + 1])
                nc.sync.dma_start(out=gin[b * 128:b * 128 + 128, :],
                                  in_=g2[:, :])
            nc.gpsimd.collective_compute(
                "AllGather", mybir.AluOpType.bypass,
                replica_groups=[list(range(NCORES))],
                ins=[gin[:, :]], outs=[gout[:, :]],
            )

            # ---------------- Layer 2: agg(t2) -> h2 -> t3 ------------------
            for b in range(BLK):
                acc = agg_block(b, 128, swapped=True)
                p2 = hpool.tile([128, 128], dt.bfloat16, tag="p1b")
                nc.scalar.activation(p2[:, :], acc[:, :],
                                     mybir.ActivationFunctionType.Relu)
                acc3 = ppt.tile([128, 128], dt.float32, tag="tp")
                nc.tensor.matmul(acc3[:, 0:16], p2[:, :], w3c[:, :],
                                 start=True, stop=False)
                nc.tensor.matmul(acc3[:, 0:16],
                                 sdb[0:1, b * 128:b * 128 + 128],
                                 sdb[0:1, SDW + 128:SDW + 144], start=False,
                                 stop=True)
                g3 = gpool.tile([128, 16], dt.bfloat16, tag="g3")
                nc.vector.tensor_scalar_mul(g3[:, :], acc3[:, 0:16],
                                            dinv2_t[:, b:b + 1])
                nc.sync.dma_start(out=gin[b * 128:b * 128 + 128, 0:16],
                                  in_=g3[:, :])
            nc.gpsimd.collective_compute(
                "AllGather", mybir.AluOpType.bypass,
                replica_groups=[list(range(NCORES))],
                ins=[gin[:, :]], outs=[gout[:, :]],
            )

            # ---------------- Layer 3: agg(t3) -> sigmoid -> R-max ----------
            for b in range(BLK):
                acc = agg_block(b, 16, swapped=False)
                sg = hpool.tile([128, 16], dt.float32, tag="sg")
                nc.scalar.activation(sg[:, :], acc[:, 0:16],
                                     mybir.ActivationFunctionType.Sigmoid,
                                     scale=dinv_t[:, b:b + 1])
                tmp = hpool.tile([128, C, C], dt.float32, tag="tmp")
                nc.vector.tensor_tensor(
                    out=tmp[:, :, :],
                    in0=sg[:, 0:C].unsqueeze(1).broadcast_to([128, C, C]),
                    in1=r_t[:, :].rearrange("p (a b) -> p a b", a=C),
                    op=mybir.AluOpType.mult,
                )
                o13 = gpool.tile([128, C], dt.float32, tag="o13")
                nc.vector.tensor_reduce(o13[:, :], tmp[:, :, :],
                                        axis=mybir.AxisListType.X,
                                        op=mybir.AluOpType.max)
                rows = 128 if b < BLK - 1 else LASTB
                nc.sync.dma_start(out=out[b * 128:b * 128 + rows, :],
                                  in_=o13[0:rows, :])

    nc.compile()
    return nc


def kernel(x, edge_index, R, W1, b1, W2, b2, W3, b3, **_):
    global LAST_RESULTS
    import concourse.mybir  # noqa: F401
    from concourse.bass_utils import run_bass_kernel_spmd

    x = np.asarray(x, np.float32)
    edge_index = np.asarray(edge_index, np.int32)
    deg, TL, off, TOT, gidx = _prep_edges(edge_index)

    nc = _build_program(TL, off, TOT)

    TT = TOT[0] + TOT[1]
    SDW = BLK * 128
    C_IO = TT
    C_W2 = C_IO + 128
    C_W3 = C_W2 + 128
    C_WK = C_W3 + 16
    C_SD = C_WK + 128
    CB = C_SD + SDW + 144
    F_D2 = BLK
    F_R = 2 * BLK
    F_ID = F_R + C * C
    FB = F_ID + 128

    dinv = 1.0 / np.sqrt(deg)
    dinv2 = dinv * dinv
    sqrtdeg = np.sqrt(deg)

    W1 = np.asarray(W1, np.float32)
    W2 = np.asarray(W2, np.float32)
    W3 = np.asarray(W3, np.float32)
    b1 = np.asarray(b1, np.float32)
    b2 = np.asarray(b2, np.float32)
    b3 = np.asarray(b3, np.float32)
    R = np.asarray(R, np.float32)

    # shared bf16 blob pieces
    iota = np.tile(np.arange(128, dtype=np.float32), (128, 1))
    w2cols = W2.T                                   # [H(in f), H(out o)]
    w3cols = np.zeros((128, 16), np.float32)
    w3cols[:, :C] = W3.T
    wkm = np.zeros((128, 128), np.float32)
    wkm[0:DIN, :] = W1.T
    wkm[DIN, :] = b1

    in_maps = []
    for k in range(NCORES):
        nd = slice(k * SH, (k + 1) * SH)
        x128 = np.zeros((CH, H), bf16)
        x128[:SH, 0:DIN] = (x[nd] * dinv[nd][:, None]).astype(bf16)
        x128[:SH, DIN] = dinv[nd].astype(bf16)

        crb = np.zeros((128, CB), bf16)
        gi_np, cr_np = gidx[k]
        crb[:, 0:TT] = cr_np
        crb[:, C_IO:C_IO + 128] = iota.astype(bf16)
        crb[:, C_W2:C_W2 + 128] = w2cols.astype(bf16)
        crb[:, C_W3:C_W3 + 16] = w3cols.astype(bf16)
        crb[:, C_WK:C_WK + 128] = wkm.astype(bf16)
        crb[0, C_SD:C_SD + SH] = sqrtdeg[nd].astype(bf16)
        crb[0, C_SD + SDW:C_SD + SDW + 128] = b2.astype(bf16)
        crb[0, C_SD + SDW + 128:C_SD + SDW + 128 + C] = b3.astype(bf16)

        f32v = np.zeros((128, FB), np.float32)
        dloc = np.ones(BLK * 128, np.float32)
        dloc[:SH] = dinv[nd]
        f32v[:, 0:BLK] = dloc.reshape(BLK, 128).T
        d2loc = np.ones(BLK * 128, np.float32)
        d2loc[:SH] = dinv2[nd]
        f32v[:, F_D2:F_D2 + BLK] = d2loc.reshape(BLK, 128).T
        f32v[:, F_R:F_R + C * C] = np.tile(R.reshape(1, C * C), (128, 1))
        f32v[:, F_ID:F_ID + 128] = np.eye(128, dtype=np.float32)

        in_maps.append({"x128": x128, "gi": gi_np, "crb": crb, "f32b": f32v})

    trace = os.environ.get("GNN_TRACE") == "1"
    res = run_bass_kernel_spmd(nc, in_maps, core_ids=list(range(NCORES)),
                               trace=trace)
    LAST_RESULTS = res

    reps = int(os.environ.get("GNN_BENCH", "0"))
    if reps > 0:
        _bench(nc, in_maps, reps)
    return np.concatenate([res.results[k]["out"] for k in range(NCORES)],
                          axis=0)


BENCH_TIMES = None
BENCH_PIPELINED_NS = None


def _bench(nc, in_maps, reps):
    """Time repeated executions of the already-built program through a single
    jit instance (NEFF compile amortized away; inputs device_put once)."""
    global BENCH_TIMES
    import time
    import jax
    from jax.sharding import Mesh, PartitionSpec, NamedSharding
    from jax.experimental.shard_map import shard_map
    import concourse.mybir as mybir
    from concourse.bass2jax import (_bass_exec_p, partition_id_tensor,
                                    install_neuronx_cc_hook)

    install_neuronx_cc_hook()
    in_names, out_names, out_avals, zero_outs = [], [], [], []
    pname = nc.partition_id_tensor.name if nc.partition_id_tensor else None
    for alloc in nc.m.functions[0].allocations:
        if not isinstance(alloc, mybir.MemoryLocationSet):
            continue
        name = alloc.memorylocations[0].name
        if alloc.kind == "ExternalInput":
            if name != pname:
                in_names.append(name)
        elif alloc.kind == "ExternalOutput":
            out_names.append(name)
            shape = tuple(alloc.tensor_shape)
            dtype = mybir.dt.np(alloc.dtype)
            out_avals.append(jax.core.ShapedArray(shape, dtype))
            zero_outs.append(np.zeros(shape, dtype))
    n_params = len(in_names)
    all_names = in_names + out_names + ([pname] if pname else [])

    def _body(*args):
        ops = list(args)
        if pname:
            ops.append(partition_id_tensor())
        return tuple(_bass_exec_p.bind(
            *ops, out_avals=tuple(out_avals), in_names=tuple(all_names),
            out_names=tuple(out_names), lowering_input_output_aliases=(),
            sim_require_finite=True, sim_require_nnan=True, nc=nc))

    devices = jax.devices()[:NCORES]
    mesh = Mesh(np.asarray(devices), ("core",))
    nouts = len(out_names)
    sharded = jax.jit(
        shard_map(_body, mesh=mesh,
                  in_specs=(PartitionSpec("core"),) * (n_params + nouts),
                  out_specs=(PartitionSpec("core"),) * nouts, check_rep=False),
        donate_argnums=tuple(range(n_params, n_params + nouts)),
        keep_unused=True)
    sh = NamedSharding(mesh, PartitionSpec("core"))
    dev_in = [jax.device_put(
        np.concatenate([np.asarray(in_maps[c][nm]) for c in range(NCORES)],
                       axis=0), sh)
        for nm in in_names]
    jax.block_until_ready(dev_in)
    times = []
    for i in range(reps + 1):
        zs = [jax.device_put(
            np.zeros((NCORES * z.shape[0], *z.shape[1:]), z.dtype), sh)
            for z in zero_outs]
        t0 = time.perf_counter()
        outs = sharded(*dev_in, *zs)
        jax.block_until_ready(outs)
        times.append(time.perf_counter() - t0)
    BENCH_TIMES = times
    print("bench wall times (s):", " ".join(f"{t:.4f}" for t in times))
    print(f"bench min/median after warmup: {min(times[1:]):.4f} / "
          f"{sorted(times[1:])[len(times[1:]) // 2]:.4f}")

    # pipelined async dispatch: amortizes per-call RPC overhead
    NPIPE = 96
    best = None
    zss = [[jax.device_put(
        np.zeros((NCORES * z.shape[0], *z.shape[1:]), z.dtype), sh)
        for z in zero_outs] for _ in range(NPIPE)]
    for trial in range(2):
        jax.block_until_ready(zss)
        t0 = time.perf_counter()
        outs = None
        for i in range(NPIPE):
            outs = sharded(*dev_in, *zss[i])
        jax.block_until_ready(outs)
        tp = (time.perf_counter() - t0) / NPIPE
        best = tp if best is None or tp < best else best
        zss = [[jax.device_put(
            np.zeros((NCORES * z.shape[0], *z.shape[1:]), z.dtype), sh)
            for z in zero_outs] for _ in range(NPIPE)]
    global BENCH_PIPELINED_NS
    BENCH_PIPELINED_NS = int(best * 1e9)
    print(f"bench pipelined per-exec: {best * 1e3:.3f} ms "
          f"({best * 1e9:.0f} ns upper bound)")
